# revision 10
# baseline (speedup 1.0000x reference)
"""Attention (B=4, S=4096, W=512, E=64) on 8 TRN2 NeuronCores.

Sharding: core c handles batch b = c//2, query half h = c%2 (2048 queries).
Each core receives x[b]^T as bf16 with the key/value columns ordered so that
this core's query half occupies columns [0, 2048) (softmax over keys is
permutation invariant as long as K and V share the order, so odd cores get
the two halves swapped). K/V are computed for the full sequence locally; a
flash-style attention runs over the core's query half. No collectives.

The schedule is built around the measured wall: every score element must
cross PSUM->SBUF through ScalarE or VectorE (exp), at 1 elem/cycle/lane
for fp32 reads -- ~1.1us per 2-ktile pair on ScalarE, ~1.2us on VectorE.
So:
  * qc0's attention is WOVEN INTO the projection loop: during the
    DMA-bound x stream (~7-26us) both exp engines and the PE run qc0's
    score/exp/AV work that would otherwise serialize after it.
  * each k-pair's exp is SPLIT across both engines (ka on one, kb on the
    other, picked by an emission-time load counter) so the PSUM score
    ring recycles after ~0.7us instead of ~1.3us.
  * the post-weave loop (qc1-3) emits blocks of 3 score pairs then the 3
    deferred fp8 DoubleRow AV matmuls, amortizing the normal<->DoubleRow
    weight-buffer flush (~200ns per switch).
  * K^T replication DMAs ride the gpsimd SWDGE queue: on the sync/scalar
    HWDGE queues their bias-add waits would stall the x stream.
  * x loads alternate t0/t1 on sync, t2/t3 on scalar, weights first, so
    both 4-deep DMA semaphore pipelines stay full.
  * a burst of identity matmuls warms the PE HAM clock gate before the
    first projection.

Per-core dataflow (fp32 scores in PSUM, fp8e4 P, DoubleRow AV):
  x^T [512,4096] --[Wv|Wk]--> kv = V^T (p0:64) / K^T (p64:128)
  x^T[:, :2048] --[Wq|Wq]--> Q^T duplicated on both partition halves
  V' = [V | 1 | pad-to-80] fp8 tiles via PE transpose (4 per chunk into
  one PSUM tile, one combined copy out)
  scores S^T[k,q] = K^T.T @ Q^T, two 64-row PE groups concurrent
  P = exp(S^T/8) -> fp8e4: ScalarE native Exp / VectorE Schraudolph
  (bits = s*SCH_A + SCH_B as int8 IS the e4m3 encoding of exp(s/8))
  Z'^T[e+1,q] += V'.T @ P, one DoubleRow matmul per pair (ones column
  accumulates the softmax denominator)
  normalize per 512-query chunk: 4 PE transposes into one PSUM tile,
  batched reciprocal, scale, DMA out.
"""

import numpy as np
import ml_dtypes

import concourse.bass as bass
import concourse.mybir as mybir
import concourse.tile as tile
from concourse import bacc
from concourse.bass import ts
from concourse.masks import make_identity
from concourse.bass_utils import run_bass_kernel_spmd

BF16 = mybir.dt.bfloat16
F32 = mybir.dt.float32
FP8 = mybir.dt.float8e4
INT8 = mybir.dt.int8
NP_BF16 = ml_dtypes.bfloat16

SCH_A = 0.125 * 8 * 1.4426950408889634
SCH_B = 56.0 - 0.458

B = 4
S_FULL = 4096
W = 512
E = 64
TQ = 2048
WT = W // 128
KT = S_FULL // 128
KP = KT // 2
QC = TQ // 512
NCH = S_FULL // 512
SCALE = 0.125
BLK = 3
N_WARM = 32

_NC_CACHE = {}


def build_nc():
    nc = bacc.Bacc("TRN2", target_bir_lowering=False)
    xT = nc.dram_tensor("xT", [W, S_FULL], BF16, kind="ExternalInput")
    wqq = nc.dram_tensor("wqq", [W, 128], BF16, kind="ExternalInput")
    wkv = nc.dram_tensor("wkv", [W, 128], BF16, kind="ExternalInput")
    bqq = nc.dram_tensor("bqq", [128, 1], F32, kind="ExternalInput")
    bkv = nc.dram_tensor("bkv", [128, 1], F32, kind="ExternalInput")
    y = nc.dram_tensor("y", [TQ, E], F32, kind="ExternalOutput")

    # emission-time engine load balancing (scalar vs vector), in ns
    load = {"se": 0.0, "dve": 0.0}

    def pick(se_cost, dve_cost):
        """Pick the engine that finishes this op sooner; update counters."""
        if load["se"] + se_cost <= load["dve"] + dve_cost:
            load["se"] += se_cost
            return "se"
        load["dve"] += dve_cost
        return "dve"

    with tile.TileContext(nc) as tc:
        with (
            tc.tile_pool(name="const", bufs=1) as const,
            tc.tile_pool(name="psZ", bufs=1, space="PSUM") as psZ,
            tc.tile_pool(name="pp", bufs=10) as ppool,
            tc.tile_pool(name="zsb", bufs=2) as zsbp,
            tc.tile_pool(name="small", bufs=2) as small,
            tc.tile_pool(name="outp", bufs=2) as outp,
        ):
            wqq_sb = const.tile([128, WT, 128], BF16)
            wkv_sb = const.tile([128, WT, 128], BF16)
            bqq_sb = const.tile([128, 1], F32)
            bkv_sb = const.tile([128, 1], F32)
            xt_sb = const.tile([128, WT, S_FULL], BF16)

            nc.scalar.dma_start(
                out=wkv_sb, in_=wkv[:, :].rearrange("(t p) m -> p t m", t=WT)
            )
            nc.scalar.dma_start(out=bkv_sb, in_=bkv[:, :])
            nc.scalar.dma_start(
                out=wqq_sb, in_=wqq[:, :].rearrange("(t p) m -> p t m", t=WT)
            )
            nc.scalar.dma_start(out=bqq_sb, in_=bqq[:, :])
            for ch2 in range(NCH // 2):
                for t in range(WT):
                    eng = nc.sync if t < 2 else nc.scalar
                    eng.dma_start(
                        out=xt_sb[:, t, ts(ch2, 1024)],
                        in_=xT[t * 128:(t + 1) * 128, ts(ch2, 1024)],
                    )

            ident_bf = const.tile([64, 64], BF16)
            make_identity(nc, ident_bf)
            ident_f32 = const.tile([E + 1, E + 1], F32)
            make_identity(nc, ident_f32)

            kv_sb = const.tile([128, S_FULL], BF16)
            krep = const.tile([64, S_FULL], BF16)
            qtpair = const.tile([128, TQ], BF16)
            vp_sb = const.tile([128, KT, 80], FP8)  # V' = [V | 1 | pad]
            nc.vector.memset(vp_sb, 1.0)

            # ---- attention state shared by the weave and the main loop
            att = {"zp": None, "pending": [], "kp": 0}

            def av_flush(zp, n=None):
                todo = att["pending"] if n is None else att["pending"][:n]
                att["pending"] = att["pending"][len(todo):]
                for pp_, pka in todo:
                    nc.tensor.matmul(
                        zp, vp_sb[:, pka:pka + 2, 0:E + 1], pp_[:, :, :],
                        start=(pka == 0), stop=(pka == KT - 2),
                        perf_mode=mybir.MatmulPerfMode.DoubleRow,
                    )

            def emit_exp_half(src_ap, dst_ap):
                # src: [128, 512] fp32 PSUM slice; dst: [128, 512] fp8 SBUF
                if pick(620.0, 660.0) == "se":
                    nc.scalar.activation(
                        dst_ap, src_ap,
                        mybir.ActivationFunctionType.Exp, scale=SCALE,
                    )
                else:
                    nc.vector.tensor_scalar(
                        dst_ap.bitcast(INT8), src_ap, SCH_A, SCH_B,
                        op0=mybir.AluOpType.mult, op1=mybir.AluOpType.add,
                    )

            # ================= phase 1: projections woven with qc0 =======
            with tc.tile_pool(name="psA", bufs=6, space="PSUM") as psA:
                warm = psA.tile([64, 64], F32, tag="mm", name="warm")
                for wi in range(N_WARM):
                    nc.tensor.matmul(
                        warm, ident_bf, ident_bf,
                        start=(wi == 0), stop=(wi == N_WARM - 1),
                    )

                def emit_kv_proj(ch):
                    ps = psA.tile([128, 512], F32, tag="mm", name=f"pskv{ch}")
                    for t in range(WT):
                        nc.tensor.matmul(
                            ps, wkv_sb[:, t, :], xt_sb[:, t, ts(ch, 512)],
                            start=(t == 0), stop=(t == WT - 1),
                        )
                    nc.vector.tensor_scalar_add(
                        kv_sb[:, ts(ch, 512)], ps, bkv_sb
                    )
                    load["dve"] += 550.0
                    # SWDGE: the bias wait must not stall the HWDGE queues
                    nc.gpsimd.dma_start(
                        out=krep[:, ts(ch, 512)], in_=kv_sb[64:128, ts(ch, 512)]
                    )

                def emit_vtrans4(ch):
                    # 4 transposes into one PSUM tile, one combined copy
                    vt_ps = psA.tile(
                        [128, 4, E], BF16, tag="vt", bufs=1, name=f"vtps{ch}"
                    )
                    for j in range(4):
                        nc.tensor.transpose(
                            vt_ps[:, j, :], kv_sb[0:64, ts(4 * ch + j, 128)],
                            ident_bf,
                        )
                    if pick(600.0, 330.0) == "se":
                        nc.scalar.copy(vp_sb[:, 4 * ch:4 * ch + 4, 0:E], vt_ps)
                    else:
                        nc.vector.tensor_copy(
                            vp_sb[:, 4 * ch:4 * ch + 4, 0:E], vt_ps
                        )

                def emit_q_chunk(ch):
                    psq = psA.tile([128, 512], F32, tag="mm", name=f"psq{ch}")
                    for t in range(WT):
                        nc.tensor.matmul(
                            psq, wqq_sb[:, t, :], xt_sb[:, t, ts(ch, 512)],
                            start=(t == 0), stop=(t == WT - 1),
                        )
                    nc.scalar.activation(
                        qtpair[:, ts(ch, 512)], psq,
                        mybir.ActivationFunctionType.Identity, bias=bqq_sb,
                    )
                    load["se"] += 320.0

                def weave_att_kp():
                    # one qc0 k-pair: two single-ktile score MMs (psA ring),
                    # exp halves on both engines, deferred AV
                    kp = att["kp"]
                    att["kp"] += 1
                    ka, kb = 2 * kp, 2 * kp + 1
                    sa = psA.tile([128, 512], F32, tag="mm", name=f"wsa{kp}")
                    sb_ = psA.tile([128, 512], F32, tag="mm", name=f"wsb{kp}")
                    nc.tensor.matmul(
                        sa, krep[:, ts(ka, 128)], qtpair[0:64, 0:512],
                        start=True, stop=True,
                    )
                    nc.tensor.matmul(
                        sb_, kv_sb[64:128, ts(kb, 128)], qtpair[64:128, 0:512],
                        start=True, stop=True,
                    )
                    av_flush(att["zp"], n=min(len(att["pending"]), 1))
                    p_sb = ppool.tile(
                        [128, 2, 512], FP8, tag="p", name=f"wp{kp}"
                    )
                    emit_exp_half(sa, p_sb[:, 0, :])
                    emit_exp_half(sb_, p_sb[:, 1, :])
                    att["pending"].append((p_sb, ka))

                att["zp"] = psZ.tile([E + 1, 512], F32, tag="zacc", name="zacc0")

                emit_kv_proj(0)
                emit_q_chunk(0)
                emit_kv_proj(1)
                emit_vtrans4(0)
                emit_q_chunk(1)
                for ch in range(2, NCH):
                    emit_kv_proj(ch)
                    emit_vtrans4(ch - 1)
                    if ch < 4:
                        emit_q_chunk(ch)
                    weave_att_kp()  # kp = ch-2 (needs krep/vp of chunk kp//2)
                    if ch >= 3:
                        weave_att_kp()
                emit_vtrans4(7)
                while att["kp"] < KP:  # qc0 kps 11..15
                    weave_att_kp()

            # ================= phase 2: qc1-3 blocked loop ===============
            with (
                tc.tile_pool(name="psB", bufs=3, space="PSUM") as psB,
                tc.tile_pool(name="psN", bufs=1, space="PSUM") as psN,
            ):
                norm_state = {}

                def norm_start(qc, zp):
                    zsb = zsbp.tile(
                        [E + 1, 512], F32, tag="zsb", name=f"zsb{qc}"
                    )
                    nc.vector.tensor_copy(zsb[:, 0:256], zp[:, 0:256])
                    nc.scalar.copy(zsb[:, 256:512], zp[:, 256:512])
                    load["dve"] += 390.0
                    load["se"] += 390.0
                    o_sb = outp.tile([128, 4, E], F32, tag="o", name=f"osb{qc}")
                    zt = psN.tile(
                        [128, 4, E + 1], F32, tag="zt", name=f"zt{qc}"
                    )
                    r = small.tile([128, 4], F32, tag="r", name=f"r{qc}")
                    norm_state[qc] = (zsb, o_sb, zt, r)

                def norm_sub(qc, sub, tail=False):
                    zsb, o_sb, zt, r = norm_state[qc]
                    nc.tensor.transpose(
                        zt[:, sub, :], zsb[:, ts(sub, 128)], ident_f32
                    )
                    if sub == 3 or tail:
                        lo = sub if tail else 0
                        nc.vector.reciprocal(
                            r[:, lo:sub + 1], zt[:, lo:sub + 1, E:E + 1]
                        )
                        load["dve"] += 170.0
                    if not tail:
                        return
                    if sub % 2 == 1:
                        nc.vector.tensor_scalar_mul(
                            o_sb[:, sub, :], zt[:, sub, 0:E], r[:, sub:sub + 1]
                        )
                    else:
                        nc.scalar.mul(
                            o_sb[:, sub, :], zt[:, sub, 0:E], r[:, sub:sub + 1]
                        )
                    q_eng = nc.sync if sub % 2 == 0 else nc.scalar
                    q_eng.dma_start(
                        out=y[ts(4 * qc + sub, 128), :], in_=o_sb[:, sub, :]
                    )

                def norm_muls(qc):
                    zsb, o_sb, zt, r = norm_state[qc]
                    for sub in range(4):
                        if pick(330.0, 300.0) == "se":
                            nc.scalar.mul(
                                o_sb[:, sub, :], zt[:, sub, 0:E],
                                r[:, sub:sub + 1],
                            )
                        else:
                            nc.vector.tensor_scalar_mul(
                                o_sb[:, sub, :], zt[:, sub, 0:E],
                                r[:, sub:sub + 1],
                            )
                        nc.sync.dma_start(
                            out=y[ts(4 * qc + sub, 128), :], in_=o_sb[:, sub, :]
                        )

                prev = (0, att["zp"])  # qc0 normalized during qc1
                for qc in range(1, QC):
                    zp = psZ.tile(
                        [E + 1, 512], F32, tag="zacc", name=f"zacc{qc}"
                    )
                    pending = []
                    for blk_lo in range(0, KP, BLK):
                        kps = range(blk_lo, min(blk_lo + BLK, KP))
                        sps = []
                        for kp in kps:
                            ka, kb = 2 * kp, 2 * kp + 1
                            sp = psB.tile(
                                [128, 2, 512], F32, tag="spair",
                                name=f"sp{qc}_{kp}",
                            )
                            nc.tensor.matmul(
                                sp[:, 0, :], krep[:, ts(ka, 128)],
                                qtpair[0:64, ts(qc, 512)],
                                start=True, stop=True,
                            )
                            nc.tensor.matmul(
                                sp[:, 1, :], kv_sb[64:128, ts(kb, 128)],
                                qtpair[64:128, ts(qc, 512)],
                                start=True, stop=True,
                            )
                            sps.append((kp, sp))
                        if qc == 1 and blk_lo == 0:
                            # qc0's woven tail drains into its accumulator
                            av_flush(att["zp"])
                        for pp_, pka in pending:
                            nc.tensor.matmul(
                                zp, vp_sb[:, pka:pka + 2, 0:E + 1],
                                pp_[:, :, :],
                                start=(pka == 0), stop=(pka == KT - 2),
                                perf_mode=mybir.MatmulPerfMode.DoubleRow,
                            )
                        pending = []
                        for kp, sp in sps:
                            p_sb = ppool.tile(
                                [128, 2, 512], FP8, tag="p",
                                name=f"p{qc}_{kp}",
                            )
                            emit_exp_half(sp[:, 0, :], p_sb[:, 0, :])
                            emit_exp_half(sp[:, 1, :], p_sb[:, 1, :])
                            pending.append((p_sb, 2 * kp))
                        bi = blk_lo // BLK
                        if bi == 0:
                            norm_start(*prev)
                        elif bi <= 4:
                            norm_sub(prev[0], bi - 1)
                            if bi == 4:
                                norm_muls(prev[0])
                    for pp_, pka in pending:
                        nc.tensor.matmul(
                            zp, vp_sb[:, pka:pka + 2, 0:E + 1], pp_[:, :, :],
                            start=(pka == 0), stop=(pka == KT - 2),
                            perf_mode=mybir.MatmulPerfMode.DoubleRow,
                        )
                    prev = (qc, zp)
                norm_start(*prev)
                for sub in range(4):
                    norm_sub(prev[0], sub, tail=True)
    nc.compile()
    return nc


def get_nc():
    if "nc" not in _NC_CACHE:
        _NC_CACHE["nc"] = build_nc()
    return _NC_CACHE["nc"]


def make_in_maps(x, Wq, bq, Wk, bk, Wv, bv):
    x = np.asarray(x, dtype=np.float32)
    Wq = np.asarray(Wq, dtype=np.float32)
    Wk = np.asarray(Wk, dtype=np.float32)
    Wv = np.asarray(Wv, dtype=np.float32)
    bq = np.asarray(bq, dtype=np.float32)
    bk = np.asarray(bk, dtype=np.float32)
    bv = np.asarray(bv, dtype=np.float32)

    wkv_host = np.ascontiguousarray(
        np.concatenate([Wv.T, Wk.T], axis=1)
    ).astype(NP_BF16)
    wqq_host = np.ascontiguousarray(
        np.concatenate([Wq.T, Wq.T], axis=1)
    ).astype(NP_BF16)
    bkv_host = np.ascontiguousarray(
        np.concatenate([bv, bk]).reshape(128, 1)
    ).astype(np.float32)
    bqq_host = np.ascontiguousarray(
        np.concatenate([bq, bq]).reshape(128, 1)
    ).astype(np.float32)

    in_maps = []
    for c in range(8):
        b, h = c // 2, c % 2
        xT_b = np.asarray(x[b].T, dtype=NP_BF16)
        if h == 1:
            xT_b = np.concatenate([xT_b[:, TQ:], xT_b[:, :TQ]], axis=1)
        in_maps.append(
            {
                "xT": np.ascontiguousarray(xT_b),
                "wqq": wqq_host,
                "wkv": wkv_host,
                "bqq": bqq_host,
                "bkv": bkv_host,
            }
        )
    return in_maps


def assemble(results):
    out = np.empty((B, S_FULL, E), dtype=np.float32)
    for c in range(8):
        b, h = c // 2, c % 2
        out[b, h * TQ:(h + 1) * TQ, :] = results[c]["y"]
    return out


def kernel(x, Wq, bq, Wk, bk, Wv, bv, **_unused):
    in_maps = make_in_maps(x, Wq, bq, Wk, bk, Wv, bv)
    nc = get_nc()
    res = run_bass_kernel_spmd(nc, in_maps, core_ids=list(range(8)))
    return assemble(res.results)


# revision 15
# speedup vs baseline: 1.1618x; 1.1618x over previous
"""Attention (B=4, S=4096, W=512, E=64) on 8 TRN2 NeuronCores.

Sharding: core c handles batch b = c//2, query half h = c%2 (2048 queries).
Each core receives x[b]^T as bf16 with the key/value columns ordered so that
this core's query half occupies columns [0, 2048) (softmax over keys is
permutation invariant as long as K and V share the order, so odd cores get
the two halves swapped). K/V are computed for the full sequence locally; a
flash-style attention runs over the core's query half. No collectives.

The schedule is built around the measured wall: every score element must
cross PSUM->SBUF through ScalarE or VectorE (exp), at 1 elem/cycle/lane
for fp32 reads -- ~1.1us per 2-ktile pair on ScalarE, ~1.2us on VectorE.
So:
  * qc0's attention is WOVEN INTO the projection loop: during the
    DMA-bound x stream (~7-26us) both exp engines and the PE run qc0's
    score/exp/AV work that would otherwise serialize after it.
  * each k-pair's exp is SPLIT across both engines (ka on one, kb on the
    other, picked by an emission-time load counter) so the PSUM score
    ring recycles after ~0.7us instead of ~1.3us.
  * the post-weave loop (qc1-3) emits blocks of 3 score pairs then the 3
    deferred fp8 DoubleRow AV matmuls, amortizing the normal<->DoubleRow
    weight-buffer flush (~200ns per switch).
  * K^T replication DMAs ride the gpsimd SWDGE queue: on the sync/scalar
    HWDGE queues their bias-add waits would stall the x stream.
  * x loads alternate t0/t1 on sync, t2/t3 on scalar, weights first, so
    both 4-deep DMA semaphore pipelines stay full.
  * a burst of identity matmuls warms the PE HAM clock gate before the
    first projection.

Per-core dataflow (fp32 scores in PSUM, fp8e4 P, DoubleRow AV):
  x^T [512,4096] --[Wv|Wk]--> kv = V^T (p0:64) / K^T (p64:128)
  x^T[:, :2048] --[Wq|Wq]--> Q^T duplicated on both partition halves
  V' = [V | 1 | pad-to-80] fp8 tiles via PE transpose (4 per chunk into
  one PSUM tile, one combined copy out)
  scores S^T[k,q] = K^T.T @ Q^T, two 64-row PE groups concurrent
  P = exp(S^T/8) -> fp8e4: ScalarE native Exp / VectorE Schraudolph
  (bits = s*SCH_A + SCH_B as int8 IS the e4m3 encoding of exp(s/8))
  Z'^T[e+1,q] += V'.T @ P, one DoubleRow matmul per pair (ones column
  accumulates the softmax denominator)
  normalize per 512-query chunk: 4 PE transposes into one PSUM tile,
  batched reciprocal, scale, DMA out.
"""

import numpy as np
import ml_dtypes

import concourse.bass as bass
import concourse.mybir as mybir
import concourse.tile as tile
from concourse import bacc
from concourse.bass import ts
from concourse.masks import make_identity
from concourse.bass_utils import run_bass_kernel_spmd

BF16 = mybir.dt.bfloat16
F32 = mybir.dt.float32
FP8 = mybir.dt.float8e4
INT8 = mybir.dt.int8
NP_BF16 = ml_dtypes.bfloat16

SCH_A = 0.125 * 8 * 1.4426950408889634
SCH_B = 56.0 - 0.458

B = 4
S_FULL = 4096
W = 512
E = 64
TQ = 2048
WT = W // 128
KT = S_FULL // 128
KP = KT // 2
QC = TQ // 512
NCH = S_FULL // 512
SCALE = 0.125
BLK = 3
N_WARM = 32

_NC_CACHE = {}


def build_nc():
    nc = bacc.Bacc("TRN2", target_bir_lowering=False)
    xT = nc.dram_tensor("xT", [W, S_FULL], BF16, kind="ExternalInput")
    wqq = nc.dram_tensor("wqq", [W, 128], BF16, kind="ExternalInput")
    wkv = nc.dram_tensor("wkv", [W, 128], BF16, kind="ExternalInput")
    bqq = nc.dram_tensor("bqq", [128, 1], F32, kind="ExternalInput")
    bkv = nc.dram_tensor("bkv", [128, 1], F32, kind="ExternalInput")
    y = nc.dram_tensor("y", [TQ, E], F32, kind="ExternalOutput")

    # emission-time engine load balancing (scalar vs vector), in ns
    load = {"se": 0.0, "dve": 0.0}

    def pick(se_cost, dve_cost):
        """Pick the engine that finishes this op sooner; update counters."""
        if load["se"] + se_cost <= load["dve"] + dve_cost:
            load["se"] += se_cost
            return "se"
        load["dve"] += dve_cost
        return "dve"

    with tile.TileContext(nc) as tc:
        with (
            tc.tile_pool(name="const", bufs=1) as const,
            tc.tile_pool(name="psZ", bufs=1, space="PSUM") as psZ,
            tc.tile_pool(name="pp", bufs=10) as ppool,
            tc.tile_pool(name="zsb", bufs=2) as zsbp,
            tc.tile_pool(name="small", bufs=2) as small,
            tc.tile_pool(name="outp", bufs=2) as outp,
        ):
            wqq_sb = const.tile([128, WT, 128], BF16)
            wkv_sb = const.tile([128, WT, 128], BF16)
            bqq_sb = const.tile([128, 1], F32)
            bkv_sb = const.tile([128, 1], F32)
            xt_sb = const.tile([128, WT, S_FULL], BF16)

            nc.scalar.dma_start(
                out=wkv_sb, in_=wkv[:, :].rearrange("(t p) m -> p t m", t=WT)
            )
            nc.scalar.dma_start(out=bkv_sb, in_=bkv[:, :])
            nc.scalar.dma_start(
                out=wqq_sb, in_=wqq[:, :].rearrange("(t p) m -> p t m", t=WT)
            )
            nc.scalar.dma_start(out=bqq_sb, in_=bqq[:, :])
            for ch2 in range(NCH // 2):
                for t in range(WT):
                    eng = nc.sync if t < 2 else nc.scalar
                    eng.dma_start(
                        out=xt_sb[:, t, ts(ch2, 1024)],
                        in_=xT[t * 128:(t + 1) * 128, ts(ch2, 1024)],
                    )

            ident_bf = const.tile([64, 64], BF16)
            make_identity(nc, ident_bf)
            ident_f32 = const.tile([E + 1, E + 1], F32)
            make_identity(nc, ident_f32)

            kv_sb = const.tile([128, S_FULL], BF16)
            krep = const.tile([64, S_FULL], BF16)
            qtpair = const.tile([128, TQ], BF16)
            vp_sb = const.tile([128, KT, 80], FP8)  # V' = [V | 1 | pad]
            nc.vector.memset(vp_sb, 1.0)

            # ---- attention state shared by the weave and the main loop
            att = {"zp": None, "pending": [], "kp": 0}

            def av_flush(zp, n=None):
                todo = att["pending"] if n is None else att["pending"][:n]
                att["pending"] = att["pending"][len(todo):]
                for pp_, pka in todo:
                    nc.tensor.matmul(
                        zp, vp_sb[:, pka:pka + 2, 0:E + 1], pp_[:, :, :],
                        start=(pka == 0), stop=(pka == KT - 2),
                        perf_mode=mybir.MatmulPerfMode.DoubleRow,
                    )

            def _exp_se(src_ap, dst_ap):
                nc.scalar.activation(
                    dst_ap, src_ap,
                    mybir.ActivationFunctionType.Exp, scale=SCALE,
                )

            def _exp_dve(src_ap, dst_ap):
                nc.vector.tensor_scalar(
                    dst_ap.bitcast(INT8), src_ap, SCH_A, SCH_B,
                    op0=mybir.AluOpType.mult, op1=mybir.AluOpType.add,
                )

            def emit_exp_pair(sp, p_sb):
                # one full [128,2,512] instruction on the lighter engine
                if pick(1120.0, 1230.0) == "se":
                    _exp_se(sp[:, :, :], p_sb[:, :, :])
                else:
                    _exp_dve(sp[:, :, :], p_sb[:, :, :])

            def emit_exp_2halves(sa, sb_, p_sb):
                # weave: the pair lives in two 1-bank tiles; both halves
                # on ONE engine so the pair costs ~one engine's worth
                if pick(1160.0, 1340.0) == "se":
                    _exp_se(sa, p_sb[:, 0, :])
                    _exp_se(sb_, p_sb[:, 1, :])
                else:
                    _exp_dve(sa, p_sb[:, 0, :])
                    _exp_dve(sb_, p_sb[:, 1, :])

            # ================= phase 1: projections woven with qc0 =======
            with tc.tile_pool(name="psA", bufs=6, space="PSUM") as psA:
                warm = psA.tile([64, 64], F32, tag="mm", name="warm")
                for wi in range(N_WARM):
                    nc.tensor.matmul(
                        warm, ident_bf, ident_bf,
                        start=(wi == 0), stop=(wi == N_WARM - 1),
                    )

                def emit_kv_proj(ch):
                    ps = psA.tile([128, 512], F32, tag="mm", name=f"pskv{ch}")
                    for t in range(WT):
                        nc.tensor.matmul(
                            ps, wkv_sb[:, t, :], xt_sb[:, t, ts(ch, 512)],
                            start=(t == 0), stop=(t == WT - 1),
                        )
                    nc.vector.tensor_scalar_add(
                        kv_sb[:, ts(ch, 512)], ps, bkv_sb
                    )
                    load["dve"] += 550.0
                    # scalar queue: its x issues are done by ~13us and the
                    # bias waits land in order without stalling anything
                    # (y-output DMAs all ride sync)
                    nc.scalar.dma_start(
                        out=krep[:, ts(ch, 512)], in_=kv_sb[64:128, ts(ch, 512)]
                    )

                def emit_vtrans4(ch):
                    # 4 transposes into one PSUM tile, one combined copy
                    vt_ps = psA.tile(
                        [128, 4, E], BF16, tag="vt", bufs=1, name=f"vtps{ch}"
                    )
                    for j in range(4):
                        nc.tensor.transpose(
                            vt_ps[:, j, :], kv_sb[0:64, ts(4 * ch + j, 128)],
                            ident_bf,
                        )
                    if pick(600.0, 330.0) == "se":
                        nc.scalar.copy(vp_sb[:, 4 * ch:4 * ch + 4, 0:E], vt_ps)
                    else:
                        nc.vector.tensor_copy(
                            vp_sb[:, 4 * ch:4 * ch + 4, 0:E], vt_ps
                        )

                def emit_q_chunk(ch):
                    psq = psA.tile([128, 512], F32, tag="mm", name=f"psq{ch}")
                    for t in range(WT):
                        nc.tensor.matmul(
                            psq, wqq_sb[:, t, :], xt_sb[:, t, ts(ch, 512)],
                            start=(t == 0), stop=(t == WT - 1),
                        )
                    nc.scalar.activation(
                        qtpair[:, ts(ch, 512)], psq,
                        mybir.ActivationFunctionType.Identity, bias=bqq_sb,
                    )
                    load["se"] += 320.0

                def weave_att_kp():
                    # one qc0 k-pair: two single-ktile score MMs (psA ring),
                    # exp halves on both engines, deferred AV
                    kp = att["kp"]
                    att["kp"] += 1
                    ka, kb = 2 * kp, 2 * kp + 1
                    sa = psA.tile([128, 512], F32, tag="mm", name=f"wsa{kp}")
                    sb_ = psA.tile([128, 512], F32, tag="mm", name=f"wsb{kp}")
                    nc.tensor.matmul(
                        sa, krep[:, ts(ka, 128)], qtpair[0:64, 0:512],
                        start=True, stop=True,
                    )
                    nc.tensor.matmul(
                        sb_, kv_sb[64:128, ts(kb, 128)], qtpair[64:128, 0:512],
                        start=True, stop=True,
                    )
                    av_flush(att["zp"], n=min(len(att["pending"]), 1))
                    p_sb = ppool.tile(
                        [128, 2, 512], FP8, tag="p", name=f"wp{kp}"
                    )
                    emit_exp_2halves(sa, sb_, p_sb)
                    att["pending"].append((p_sb, ka))

                att["zp"] = psZ.tile([E + 1, 512], F32, tag="zacc", name="zacc0")

                emit_kv_proj(0)
                emit_q_chunk(0)
                emit_kv_proj(1)
                emit_vtrans4(0)
                emit_q_chunk(1)
                for ch in range(2, NCH):
                    emit_kv_proj(ch)
                    emit_vtrans4(ch - 1)
                    if ch < 4:
                        emit_q_chunk(ch)
                    weave_att_kp()  # kp = ch-2 (needs krep/vp of chunk kp//2)
                    if ch >= 3:
                        weave_att_kp()
                emit_vtrans4(7)
                while att["kp"] < KP:  # qc0 kps 11..15
                    weave_att_kp()

            # ================= phase 2: qc1-3 blocked loop ===============
            with (
                tc.tile_pool(name="psB", bufs=3, space="PSUM") as psB,
                tc.tile_pool(name="psN", bufs=1, space="PSUM") as psN,
            ):
                norm_state = {}

                def norm_start(qc, zp):
                    zsb = zsbp.tile(
                        [E + 1, 512], F32, tag="zsb", name=f"zsb{qc}"
                    )
                    nc.vector.tensor_copy(zsb[:, 0:256], zp[:, 0:256])
                    nc.scalar.copy(zsb[:, 256:512], zp[:, 256:512])
                    load["dve"] += 390.0
                    load["se"] += 390.0
                    o_sb = outp.tile([128, 4, E], F32, tag="o", name=f"osb{qc}")
                    zt = psN.tile(
                        [128, 4, E + 1], F32, tag="zt", name=f"zt{qc}"
                    )
                    r = small.tile([128, 4], F32, tag="r", name=f"r{qc}")
                    norm_state[qc] = (zsb, o_sb, zt, r)

                def norm_sub(qc, sub, tail=False):
                    zsb, o_sb, zt, r = norm_state[qc]
                    nc.tensor.transpose(
                        zt[:, sub, :], zsb[:, ts(sub, 128)], ident_f32
                    )
                    if sub == 3 or tail:
                        lo = sub if tail else 0
                        nc.vector.reciprocal(
                            r[:, lo:sub + 1], zt[:, lo:sub + 1, E:E + 1]
                        )
                        load["dve"] += 170.0
                    if not tail:
                        return
                    if sub % 2 == 1:
                        nc.vector.tensor_scalar_mul(
                            o_sb[:, sub, :], zt[:, sub, 0:E], r[:, sub:sub + 1]
                        )
                    else:
                        nc.scalar.mul(
                            o_sb[:, sub, :], zt[:, sub, 0:E], r[:, sub:sub + 1]
                        )
                    q_eng = nc.sync if sub % 2 == 0 else nc.scalar
                    q_eng.dma_start(
                        out=y[ts(4 * qc + sub, 128), :], in_=o_sb[:, sub, :]
                    )

                def norm_muls(qc):
                    zsb, o_sb, zt, r = norm_state[qc]
                    for sub in range(4):
                        if pick(330.0, 300.0) == "se":
                            nc.scalar.mul(
                                o_sb[:, sub, :], zt[:, sub, 0:E],
                                r[:, sub:sub + 1],
                            )
                        else:
                            nc.vector.tensor_scalar_mul(
                                o_sb[:, sub, :], zt[:, sub, 0:E],
                                r[:, sub:sub + 1],
                            )
                        nc.sync.dma_start(
                            out=y[ts(4 * qc + sub, 128), :], in_=o_sb[:, sub, :]
                        )

                prev = (0, att["zp"])  # qc0 normalized during qc1
                for qc in range(1, QC):
                    zp = psZ.tile(
                        [E + 1, 512], F32, tag="zacc", name=f"zacc{qc}"
                    )
                    pending = []
                    for blk_lo in range(0, KP, BLK):
                        kps = range(blk_lo, min(blk_lo + BLK, KP))
                        sps = []
                        for kp in kps:
                            ka, kb = 2 * kp, 2 * kp + 1
                            sp = psB.tile(
                                [128, 2, 512], F32, tag="spair",
                                name=f"sp{qc}_{kp}",
                            )
                            nc.tensor.matmul(
                                sp[:, 0, :], krep[:, ts(ka, 128)],
                                qtpair[0:64, ts(qc, 512)],
                                start=True, stop=True,
                            )
                            nc.tensor.matmul(
                                sp[:, 1, :], kv_sb[64:128, ts(kb, 128)],
                                qtpair[64:128, ts(qc, 512)],
                                start=True, stop=True,
                            )
                            sps.append((kp, sp))
                        if qc == 1 and blk_lo == 0:
                            # qc0's woven tail drains into its accumulator
                            av_flush(att["zp"])
                        for pp_, pka in pending:
                            nc.tensor.matmul(
                                zp, vp_sb[:, pka:pka + 2, 0:E + 1],
                                pp_[:, :, :],
                                start=(pka == 0), stop=(pka == KT - 2),
                                perf_mode=mybir.MatmulPerfMode.DoubleRow,
                            )
                        pending = []
                        for kp, sp in sps:
                            p_sb = ppool.tile(
                                [128, 2, 512], FP8, tag="p",
                                name=f"p{qc}_{kp}",
                            )
                            emit_exp_pair(sp, p_sb)
                            pending.append((p_sb, 2 * kp))
                        bi = blk_lo // BLK
                        if bi == 0:
                            norm_start(*prev)
                        elif bi <= 4:
                            norm_sub(prev[0], bi - 1)
                            if bi == 4:
                                norm_muls(prev[0])
                    for pp_, pka in pending:
                        nc.tensor.matmul(
                            zp, vp_sb[:, pka:pka + 2, 0:E + 1], pp_[:, :, :],
                            start=(pka == 0), stop=(pka == KT - 2),
                            perf_mode=mybir.MatmulPerfMode.DoubleRow,
                        )
                    prev = (qc, zp)
                norm_start(*prev)
                for sub in range(4):
                    norm_sub(prev[0], sub, tail=True)
    nc.compile()
    return nc


def get_nc():
    if "nc" not in _NC_CACHE:
        _NC_CACHE["nc"] = build_nc()
    return _NC_CACHE["nc"]


def make_in_maps(x, Wq, bq, Wk, bk, Wv, bv):
    x = np.asarray(x, dtype=np.float32)
    Wq = np.asarray(Wq, dtype=np.float32)
    Wk = np.asarray(Wk, dtype=np.float32)
    Wv = np.asarray(Wv, dtype=np.float32)
    bq = np.asarray(bq, dtype=np.float32)
    bk = np.asarray(bk, dtype=np.float32)
    bv = np.asarray(bv, dtype=np.float32)

    wkv_host = np.ascontiguousarray(
        np.concatenate([Wv.T, Wk.T], axis=1)
    ).astype(NP_BF16)
    wqq_host = np.ascontiguousarray(
        np.concatenate([Wq.T, Wq.T], axis=1)
    ).astype(NP_BF16)
    bkv_host = np.ascontiguousarray(
        np.concatenate([bv, bk]).reshape(128, 1)
    ).astype(np.float32)
    bqq_host = np.ascontiguousarray(
        np.concatenate([bq, bq]).reshape(128, 1)
    ).astype(np.float32)

    in_maps = []
    for c in range(8):
        b, h = c // 2, c % 2
        xT_b = np.asarray(x[b].T, dtype=NP_BF16)
        if h == 1:
            xT_b = np.concatenate([xT_b[:, TQ:], xT_b[:, :TQ]], axis=1)
        in_maps.append(
            {
                "xT": np.ascontiguousarray(xT_b),
                "wqq": wqq_host,
                "wkv": wkv_host,
                "bqq": bqq_host,
                "bkv": bkv_host,
            }
        )
    return in_maps


def assemble(results):
    out = np.empty((B, S_FULL, E), dtype=np.float32)
    for c in range(8):
        b, h = c // 2, c % 2
        out[b, h * TQ:(h + 1) * TQ, :] = results[c]["y"]
    return out


def kernel(x, Wq, bq, Wk, bk, Wv, bv, **_unused):
    in_maps = make_in_maps(x, Wq, bq, Wk, bk, Wv, bv)
    nc = get_nc()
    res = run_bass_kernel_spmd(nc, in_maps, core_ids=list(range(8)))
    return assemble(res.results)


# revision 19
# speedup vs baseline: 1.2349x; 1.0629x over previous
"""Attention (B=4, S=4096, W=512, E=64) on 8 TRN2 NeuronCores.

Sharding: core c handles batch b = c//2, query half h = c%2 (2048 queries).
Each core receives x[b]^T as bf16 with the key/value columns ordered so that
this core's query half occupies columns [0, 2048) (softmax over keys is
permutation invariant as long as K and V share the order, so odd cores get
the two halves swapped). K/V are computed for the full sequence locally; a
flash-style attention runs over the core's query half. No collectives
(pair-wise AllGather was measured at ~17us per op in this stack - slower
than just duplicating the K/V projection on both cores of a pair).

Per-core dataflow (bf16 scores, fp8e4 AV with DoubleRow):
  x^T [512,4096] --[Wv|Wk] pass--> kv = V^T (p0:64) / K^T (p64:128)
  x^T[:, :2048] --[Wq|Wq] pass--> Q^T duplicated on both partition halves
  K^T replicated to partitions 0:64 via SBUF->SBUF DMA on the sync
  queue (the SP engine is idle after the x loads; a gpsimd-queue DMA
  would cost a ~2us dge drain at teardown, and an Act-queue trigger
  would stall ScalarE on the bias-add semaphores)
  V' = [V | 1 | pad-to-80] tiles in fp8e4 via PE transpose, interleaved
  with the projection chunks to ride the x-DMA gaps
  scores: S^T[k,q] = K^T.T @ Q^T, two k-tiles run CONCURRENTLY in the two
  64-row PE row groups (~386ns per pair)
  P = exp(S^T/8) -> fp8e4, one full [128,2x512] instruction per k-pair,
  alternating engines: even kp native Exp on ScalarE, odd kp on VectorE
  via Schraudolph (bits = s*SCH_A + SCH_B stored int8; that bit pattern
  IS the e4m3 encoding of exp(s/8), since e4m3 has 8 codes per octave)
  Z'^T[e+1,q] += V'.T @ P as ONE fp8 DoubleRow matmul per k-pair (~405ns
  for 256 contraction rows; V' ones column accumulates the denominator)
  normalize per query chunk: PE-transpose Z'^T, reciprocal on VectorE,
  scale on ScalarE, per-sub DMA out - overlapped with the next sweep.

PSUM: projection pool 6 banks (closes before the loop; fewer buffers
starve the PE behind the bias-add/V'-copy round trips), then Z
accumulator 1 bank + norm scratch 1 bank + triple-buffered score pairs
6 banks.  AV matmuls are emitted three iterations late so the PE's
in-order stream never waits on exp; exp of pair k frees its PSUM banks
before the scores of pair k+3 need them.
"""

import numpy as np
import ml_dtypes

import concourse.bass as bass
import concourse.mybir as mybir
import concourse.tile as tile
from concourse import bacc
from concourse.bass import ts
from concourse.masks import make_identity
from concourse.bass_utils import run_bass_kernel_spmd

BF16 = mybir.dt.bfloat16
F32 = mybir.dt.float32
FP8 = mybir.dt.float8e4
INT8 = mybir.dt.int8
NP_BF16 = ml_dtypes.bfloat16

# Schraudolph exp on DVE: bits = s * SCH_A + SCH_B, stored int8, bitcast
# fp8e4.  SCH_A folds the 1/sqrt(E) softmax scale and log2(e) into the
# e4m3 exponent step (8 codes per octave); SCH_B centers on the exponent
# bias (7*8) minus the mean log error of the mantissa interpolation.
# Verified on hardware: the DVE float->int8 store rounds to nearest.
SCH_A = 0.125 * 8 * 1.4426950408889634
SCH_B = 56.0 - 0.458

B = 4
S_FULL = 4096
W = 512
E = 64
TQ = 2048  # queries per core
WT = W // 128  # 4 contraction tiles
KT = S_FULL // 128  # 32 key tiles
KP = KT // 2  # 16 key-tile pairs
QC = TQ // 512  # 4 query chunks of 512
NCH = S_FULL // 512  # 8 projection chunks
SCALE = 0.125  # 1/sqrt(E)

_NC_CACHE = {}


def build_nc():
    nc = bacc.Bacc("TRN2", target_bir_lowering=False)
    xT = nc.dram_tensor("xT", [W, S_FULL], BF16, kind="ExternalInput")
    wqq = nc.dram_tensor("wqq", [W, 128], BF16, kind="ExternalInput")
    wkv = nc.dram_tensor("wkv", [W, 128], BF16, kind="ExternalInput")
    bqq = nc.dram_tensor("bqq", [128, 1], F32, kind="ExternalInput")
    bkv = nc.dram_tensor("bkv", [128, 1], F32, kind="ExternalInput")
    y = nc.dram_tensor("y", [TQ, E], F32, kind="ExternalOutput")

    with tile.TileContext(nc) as tc:
        with (
            tc.tile_pool(name="const", bufs=1) as const,
            tc.tile_pool(name="psZ", bufs=1, space="PSUM") as psZ,
            tc.tile_pool(name="pp", bufs=8) as ppool,
            tc.tile_pool(name="zsb", bufs=2) as zsbp,
            tc.tile_pool(name="small", bufs=2) as small,
            tc.tile_pool(name="outp", bufs=2) as outp,
        ):
            # weights/biases as single HWDGE DMAs ahead of the x^T stream
            wqq_sb = const.tile([128, WT, 128], BF16)
            wkv_sb = const.tile([128, WT, 128], BF16)
            nc.scalar.dma_start(
                out=wkv_sb, in_=wkv[:, :].rearrange("(t p) m -> p t m", t=WT)
            )
            nc.scalar.dma_start(
                out=wqq_sb, in_=wqq[:, :].rearrange("(t p) m -> p t m", t=WT)
            )
            bqq_sb = const.tile([128, 1], F32)
            bkv_sb = const.tile([128, 1], F32)
            nc.scalar.dma_start(out=bkv_sb, in_=bkv[:, :])
            nc.scalar.dma_start(out=bqq_sb, in_=bqq[:, :])

            # x^T streamed in per 1024-column block (HWDGE)
            xt_sb = const.tile([128, WT, S_FULL], BF16)
            for ch2 in range(NCH // 2):
                for t in range(WT):
                    nc.sync.dma_start(
                        out=xt_sb[:, t, ts(ch2, 1024)],
                        in_=xT[t * 128:(t + 1) * 128, ts(ch2, 1024)],
                    )

            ident_bf = const.tile([64, 64], BF16)
            make_identity(nc, ident_bf)
            ident_f32 = const.tile([E + 1, E + 1], F32)
            make_identity(nc, ident_f32)

            kv_sb = const.tile([128, S_FULL], BF16)  # V^T (p0:64) / K^T (p64:)
            krep = const.tile([64, S_FULL], BF16)  # K^T replica on p0:64
            qtpair = const.tile([128, TQ], BF16)  # Q^T on both halves
            # inner dim padded 65->80 so the DoubleRow pair stride is
            # 16B-aligned
            vp_sb = const.tile([128, KT, 80], FP8)  # V' = [V | 1 | pad]
            nc.vector.memset(vp_sb, 1.0)

            with tc.tile_pool(name="psA", bufs=6, space="PSUM") as psA:
                # HAM warmup: the PE clock gate opens after ~3.4us of
                # sustained activity; burn idle pre-DMA time on tiny
                # identity matmuls so the projections run at 2.4GHz
                warm = psA.tile([64, 64], F32, tag="mm", name="warm")
                for wi in range(32):
                    nc.tensor.matmul(
                        warm, ident_bf, ident_bf,
                        start=(wi == 0), stop=(wi == 31),
                    )

                def emit_kv_proj(ch):
                    # K/V projection chunk; one fused bias add on VectorE
                    ps = psA.tile([128, 512], F32, tag="mm", name=f"pskv{ch}")
                    for t in range(WT):
                        nc.tensor.matmul(
                            ps,
                            wkv_sb[:, t, :],
                            xt_sb[:, t, ts(ch, 512)],
                            start=(t == 0),
                            stop=(t == WT - 1),
                        )
                    nc.vector.tensor_scalar_add(
                        kv_sb[:, ts(ch, 512)], ps, bkv_sb
                    )
                    nc.sync.dma_start(
                        out=krep[:, ts(ch, 512)], in_=kv_sb[64:128, ts(ch, 512)]
                    )

                def emit_vtrans1(kt_i):
                    vt_ps = psA.tile(
                        [128, E], BF16, tag="mm", name=f"vtps{kt_i}"
                    )
                    nc.tensor.transpose(
                        vt_ps, kv_sb[0:64, ts(kt_i, 128)], ident_bf
                    )
                    # PSUM bf16 -> SBUF fp8; alternate engines to balance
                    if kt_i % 2 == 0:
                        nc.vector.tensor_copy(vp_sb[:, kt_i, 0:E], vt_ps)
                    else:
                        nc.scalar.copy(vp_sb[:, kt_i, 0:E], vt_ps)

                def emit_q_chunk(ch):
                    psq = psA.tile([128, 512], F32, tag="mm", name=f"psq{ch}")
                    for t in range(WT):
                        nc.tensor.matmul(
                            psq,
                            wqq_sb[:, t, :],
                            xt_sb[:, t, ts(ch, 512)],
                            start=(t == 0),
                            stop=(t == WT - 1),
                        )
                    # Q bias on ScalarE: DVE carries the K/V biases and
                    # fp8 copies, so its projection tail otherwise delays
                    # qc0's first Schraudolph exps
                    nc.scalar.activation(
                        qtpair[:, ts(ch, 512)], psq,
                        mybir.ActivationFunctionType.Identity, bias=bqq_sb,
                    )

                emit_kv_proj(0)
                emit_q_chunk(0)
                emit_kv_proj(1)
                for kt_i in range(0, 4):
                    emit_vtrans1(kt_i)
                emit_q_chunk(1)
                emit_kv_proj(2)
                for kt_i in range(4, 8):
                    emit_vtrans1(kt_i)
                emit_q_chunk(2)
                emit_kv_proj(3)
                for kt_i in range(8, 12):
                    emit_vtrans1(kt_i)
                emit_q_chunk(3)
                for ch in range(4, NCH):
                    emit_kv_proj(ch)
                    for kt_i in range(4 * ch - 4, 4 * ch):
                        emit_vtrans1(kt_i)
                for kt_i in range(28, 32):
                    emit_vtrans1(kt_i)

            with (
                tc.tile_pool(name="psB", bufs=3, space="PSUM") as psB,
                tc.tile_pool(name="psN", bufs=1, space="PSUM") as psN,
            ):
                norm_state = {}

                def norm_start(qc, zp, tail=False):
                    zsb = zsbp.tile(
                        [E + 1, 512], F32, tag="zsb", name=f"zsb{qc}"
                    )
                    if tail:
                        # per-sub copies split across both engines so each
                        # sub's transpose chain starts as soon as its own
                        # 128 columns land (the single 686ns copy would
                        # serialize the whole tail behind it)
                        for sub in range(4):
                            if sub % 2 == 0:
                                nc.vector.tensor_copy(
                                    zsb[:, ts(sub, 128)], zp[:, ts(sub, 128)]
                                )
                            else:
                                nc.scalar.copy(
                                    zsb[:, ts(sub, 128)], zp[:, ts(sub, 128)]
                                )
                    else:
                        nc.vector.tensor_copy(zsb, zp)
                    o_sb = outp.tile([128, 4, E], F32, tag="o", name=f"osb{qc}")
                    norm_state[qc] = (zsb, o_sb)

                def norm_step(qc, sub, pool, spread=False):
                    zsb, o_sb = norm_state[qc]
                    zt = pool.tile(
                        [128, E + 1], F32, tag="zt", name=f"zt{qc}_{sub}"
                    )
                    nc.tensor.transpose(zt, zsb[:, ts(sub, 128)], ident_f32)
                    r = small.tile([128, 1], F32, tag="r", name=f"r{qc}_{sub}")
                    nc.vector.reciprocal(r, zt[:, E:E + 1])
                    if spread and sub % 2 == 1:
                        nc.vector.tensor_scalar_mul(
                            o_sb[:, sub, :], zt[:, 0:E], r
                        )
                    else:
                        nc.scalar.mul(o_sb[:, sub, :], zt[:, 0:E], r)
                    q_eng = (nc.sync, nc.scalar, nc.sync, nc.scalar)[
                        sub if spread else 0
                    ]
                    q_eng.dma_start(
                        out=y[ts(4 * qc + sub, 128), :], in_=o_sb[:, sub, :]
                    )

                prev = None  # (qc, zp) awaiting normalize
                for qc in range(QC):
                    zp = psZ.tile(
                        [E + 1, 512], F32, tag="zacc", name=f"zacc{qc}"
                    )
                    pending = []  # (p_tile, ka, kb) AVs deferred two iters
                    for kp in range(KP):
                        if prev is not None:
                            if kp == 0:
                                norm_start(*prev)
                            elif kp in (3, 6, 9, 12):
                                norm_step(prev[0], kp // 3 - 1, psN)
                        ka, kb = 2 * kp, 2 * kp + 1
                        sp = psB.tile(
                            [128, 2, 512], F32, tag="spair", name=f"sp{qc}_{kp}"
                        )
                        nc.tensor.matmul(
                            sp[:, 0, :],
                            krep[:, ts(ka, 128)],
                            qtpair[0:64, ts(qc, 512)],
                            start=True,
                            stop=True,
                        )
                        nc.tensor.matmul(
                            sp[:, 1, :],
                            kv_sb[64:128, ts(kb, 128)],
                            qtpair[64:128, ts(qc, 512)],
                            start=True,
                            stop=True,
                        )
                        p_sb = ppool.tile(
                            [128, 2, 512], FP8, tag="p", name=f"p{qc}_{kp}"
                        )
                        # full-pair exp alternates engines: ScalarE native
                        # Exp on even kp, VectorE Schraudolph on odd kp.
                        # kps 0-2 all go to ScalarE: it is idle at the qc
                        # boundary (DVE drains the Z-evacuation copy), and
                        # an early exp(kp0) completion unblocks the psB
                        # recycle that otherwise stalls the kp3 scores.
                        if kp < 3 or kp % 2 == 0:
                            nc.scalar.activation(
                                p_sb[:, :, :], sp[:, :, :],
                                mybir.ActivationFunctionType.Exp, scale=SCALE,
                            )
                        else:
                            nc.vector.tensor_scalar(
                                p_sb[:, :, :].bitcast(INT8), sp[:, :, :],
                                SCH_A, SCH_B,
                                op0=mybir.AluOpType.mult,
                                op1=mybir.AluOpType.add,
                            )
                        if len(pending) == 3:
                            pp_, pka, pkb = pending.pop(0)
                            nc.tensor.matmul(
                                zp, vp_sb[:, pka:pka + 2, 0:E + 1],
                                pp_[:, :, :],
                                start=(pka == 0), stop=False,
                                perf_mode=mybir.MatmulPerfMode.DoubleRow,
                            )
                        pending.append((p_sb, ka, kb))
                    for pp_, pka, pkb in pending:
                        nc.tensor.matmul(
                            zp, vp_sb[:, pka:pka + 2, 0:E + 1], pp_[:, :, :],
                            start=False, stop=(pkb == KT - 1),
                            perf_mode=mybir.MatmulPerfMode.DoubleRow,
                        )
                    prev = (qc, zp)
                # final chunk's normalize in the tail, per-sub pipelined
                norm_start(prev[0], prev[1], tail=True)
            with tc.tile_pool(name="psT", bufs=4, space="PSUM") as psT:
                for sub in range(4):
                    norm_step(prev[0], sub, psT, spread=True)
    nc.compile()
    return nc


def get_nc():
    if "nc" not in _NC_CACHE:
        _NC_CACHE["nc"] = build_nc()
    return _NC_CACHE["nc"]


def make_in_maps(x, Wq, bq, Wk, bk, Wv, bv):
    x = np.asarray(x, dtype=np.float32)
    Wq = np.asarray(Wq, dtype=np.float32)
    Wk = np.asarray(Wk, dtype=np.float32)
    Wv = np.asarray(Wv, dtype=np.float32)
    bq = np.asarray(bq, dtype=np.float32)
    bk = np.asarray(bk, dtype=np.float32)
    bv = np.asarray(bv, dtype=np.float32)

    wkv_host = np.ascontiguousarray(
        np.concatenate([Wv.T, Wk.T], axis=1)
    ).astype(NP_BF16)
    wqq_host = np.ascontiguousarray(
        np.concatenate([Wq.T, Wq.T], axis=1)
    ).astype(NP_BF16)
    bkv_host = np.ascontiguousarray(
        np.concatenate([bv, bk]).reshape(128, 1)
    ).astype(np.float32)
    bqq_host = np.ascontiguousarray(
        np.concatenate([bq, bq]).reshape(128, 1)
    ).astype(np.float32)

    in_maps = []
    for c in range(8):
        b, h = c // 2, c % 2
        xT_b = np.asarray(x[b].T, dtype=NP_BF16)
        if h == 1:  # put this core's query half into columns [0, 2048)
            xT_b = np.concatenate([xT_b[:, TQ:], xT_b[:, :TQ]], axis=1)
        in_maps.append(
            {
                "xT": np.ascontiguousarray(xT_b),
                "wqq": wqq_host,
                "wkv": wkv_host,
                "bqq": bqq_host,
                "bkv": bkv_host,
            }
        )
    return in_maps


def assemble(results):
    out = np.empty((B, S_FULL, E), dtype=np.float32)
    for c in range(8):
        b, h = c // 2, c % 2
        out[b, h * TQ:(h + 1) * TQ, :] = results[c]["y"]
    return out


def kernel(x, Wq, bq, Wk, bk, Wv, bv, **_unused):
    in_maps = make_in_maps(x, Wq, bq, Wk, bk, Wv, bv)
    nc = get_nc()
    res = run_bass_kernel_spmd(nc, in_maps, core_ids=list(range(8)))
    return assemble(res.results)

